# revision 21
# baseline (speedup 1.0000x reference)
"""DGCNN classifier forward pass on 8 Trainium2 NeuronCores (Bass/Tile).

Data-parallel over batch: 2 point clouds per core. Per sample:
  4 EdgeConv layers, each:
    - kNN scores via one augmented matmul: score[n,m] = <f_n,f_m> - ||f_m||^2/2
      (rank-equivalent to the reference's pairwise-distance top-k)
    - top-20 per row on the DVE via MAX8/MATCH_REPLACE cascades over
      index-packed scores (column index injected into the low 10 mantissa bits)
    - neighbor max-aggregation via GPSIMD ap_gather over u = Wn @ f, using
      monotonicity of the (positive-gamma) BN + LeakyReLU to commute max:
      h = lrelu(A*(max_k u[idx] + (Wx-Wn) @ f) + B)
  then the 1024-wide conv + max/mean pooling and the 3-layer MLP head.
"""
import numpy as np
from contextlib import ExitStack

import concourse.bass as bass
import concourse.bacc as bacc
import concourse.mybir as mybir
from concourse import tile

F32 = mybir.dt.float32
U32 = mybir.dt.uint32
U16 = mybir.dt.uint16
I16 = mybir.dt.int16
AF = mybir.ActivationFunctionType
ALU = mybir.AluOpType
AX = mybir.AxisListType

N = 1024
K = 20
EPS = 1e-5
NEG = -3.0e38
# (C, O, input location, output location) per edge-conv layer.
LAYERS = [(3, 64), (64, 64), (64, 128), (128, 256)]


def build_nc():
    nc = bacc.Bacc("TRN2", target_bir_lowering=False, debug=False)

    x_d = nc.dram_tensor("x", [2, 3, N], F32, kind="ExternalInput")
    w_d = {}
    for name, shape in [("w1", (64, 6)), ("w2", (64, 128)), ("w3", (128, 128)),
                        ("w4", (256, 256)), ("w5", (1024, 512)),
                        ("l1w", (512, 2048)), ("l2w", (256, 512)), ("l3w", (40, 256)),
                        ("l2b", (256,)), ("l3b", (40,))]:
        w_d[name] = nc.dram_tensor(name, list(shape), F32, kind="ExternalInput")
    for i, c in zip(range(1, 8), [64, 64, 128, 256, 1024, 512, 256]):
        w_d["bn%d" % i] = nc.dram_tensor("bn%d" % i, [4, c], F32, kind="ExternalInput")
    out_d = nc.dram_tensor("outT", [40, 2], F32, kind="ExternalOutput")

    with tile.TileContext(nc) as tc, ExitStack() as ctx:
        emit(nc, tc, ctx, x_d, w_d, out_d)
    nc.compile()
    return nc


def _stt_u32(nc, out, in0, imm, in1, op0, op1):
    """scalar_tensor_tensor with a uint32-typed immediate (bitwise-safe)."""
    eng = nc.vector
    return eng.add_instruction(mybir.InstTensorScalarPtr(
        name=nc.get_next_instruction_name(),
        is_scalar_tensor_tensor=True,
        op0=op0, op1=op1,
        ins=[eng.lower_ap(in0),
             mybir.ImmediateValue(dtype=U32, value=imm),
             eng.lower_ap(in1)],
        outs=[eng.lower_ap(out)],
    ))


def _ts_u32(nc, out, in0, imm, op0):
    """tensor_scalar with a uint32-typed immediate."""
    eng = nc.vector
    return eng.add_instruction(mybir.InstTensorScalarPtr(
        name=nc.get_next_instruction_name(),
        op0=op0, op1=ALU.bypass,
        ins=[eng.lower_ap(in0),
             mybir.ImmediateValue(dtype=U32, value=imm)],
        outs=[eng.lower_ap(out)],
    ))


def _bn_affine(nc, pool, bnT, tag):
    """bnT: [C<=128, 4] tile AP (cols g,b,m,v) -> (A, B) [C,1] tiles."""
    Cc = bnT.shape[0]
    A = pool.tile([Cc, 1], F32, tag=tag + "A")
    B = pool.tile([Cc, 1], F32, tag=tag + "B")
    t = pool.tile([Cc, 1], F32, tag=tag + "t")
    nc.vector.tensor_scalar(out=t[:], in0=bnT[:, 3:4], scalar1=EPS, scalar2=None,
                            op0=ALU.add)
    nc.vector.reciprocal(out=t[:], in_=t[:])
    nc.scalar.activation(out=t[:], in_=t[:], func=AF.Sqrt)
    nc.vector.tensor_tensor(out=A[:], in0=bnT[:, 0:1], in1=t[:], op=ALU.mult)
    nc.vector.tensor_tensor(out=t[:], in0=bnT[:, 2:3], in1=A[:], op=ALU.mult)
    nc.vector.tensor_tensor(out=B[:], in0=bnT[:, 1:2], in1=t[:], op=ALU.subtract)
    return A, B


def emit(nc, tc, ctx, x_d, w_d, out_d):
    wp = ctx.enter_context(tc.tile_pool(name="wp", bufs=1))
    cat = ctx.enter_context(tc.tile_pool(name="cat", bufs=1))
    work = ctx.enter_context(tc.tile_pool(name="work", bufs=2))
    feat = ctx.enter_context(tc.tile_pool(name="feat", bufs=1))
    small = ctx.enter_context(tc.tile_pool(name="small", bufs=2))
    psS = ctx.enter_context(tc.tile_pool(name="psS", bufs=2, space="PSUM"))
    psX = ctx.enter_context(tc.tile_pool(name="psX", bufs=1, space="PSUM"))
    psU = ctx.enter_context(tc.tile_pool(name="psU", bufs=1, space="PSUM"))
    gp = ctx.enter_context(tc.tile_pool(name="gp", bufs=1))
    dram = ctx.enter_context(tc.tile_pool(name="dram", bufs=1, space="DRAM"))

    # ---------------- constants ----------------
    iota = wp.tile([128, N], U32, tag="iota")
    nc.gpsimd.iota(iota[:], pattern=[[1, N]], base=0, channel_multiplier=0)
    piota = wp.tile([128, 1], U32, tag="piota")
    nc.gpsimd.iota(piota[:], pattern=[[1, 1]], base=0, channel_multiplier=1)
    fiota = wp.tile([128, 128], F32, tag="fiota")
    nc.vector.tensor_copy(out=fiota[:], in_=iota[:, 0:128])
    fpiota = wp.tile([128, 1], F32, tag="fpiota")
    nc.vector.tensor_copy(out=fpiota[:], in_=piota[:])
    I128 = wp.tile([128, 128], F32, tag="I128")
    nc.vector.tensor_scalar(out=I128[:], in0=fiota[:], scalar1=fpiota[:],
                            scalar2=None, op0=ALU.is_equal)
    ones = wp.tile([128, 1], F32, tag="ones")
    nc.vector.memset(ones[:], 1.0)
    cm05 = wp.tile([1, 128], F32, tag="cm05")
    nc.vector.memset(cm05[:], -0.5)

    # ---------------- weights (transposed loads) ----------------
    def tload(dst, src_ap):
        nc.sync.dma_start(out=dst, in_=src_ap)

    # per-layer Wn^T / Wx^T halves, each its own DMA; wxm built so every
    # instruction depends on at most one DMA (HW sync-wait slot limit).
    wn, wxm = [], []
    for nm, (O_, C2) in [("w1", (64, 6)), ("w2", (64, 128)), ("w3", (128, 128)),
                         ("w4", (256, 256))]:
        Cl = C2 // 2
        tn = wp.tile([Cl, O_], F32, tag=nm + "Tn", name=nm + "Tn")
        tload(tn[:], w_d[nm][:, 0:Cl].rearrange("o c -> c o"))
        tx = wp.tile([Cl, O_], F32, tag=nm + "Tx", name=nm + "Tx")
        tload(tx[:], w_d[nm][:, Cl:C2].rearrange("o c -> c o"))
        m = wp.tile([Cl, O_], F32, tag=nm + "wxm", name=nm + "wxm")
        nc.vector.tensor_copy(out=m[:], in_=tx[:])
        nc.vector.tensor_tensor(out=m[:], in0=m[:], in1=tn[:], op=ALU.subtract)
        wn.append(tn[:])
        wxm.append(m[:])
    w1nP = wp.tile([16, 64], F32, tag="w1nP")
    nc.vector.memset(w1nP[:], 0.0)
    nc.vector.tensor_copy(out=w1nP[0:3, :], in_=wn[0])

    # big weights: contiguous row-chunk loads + on-chip PE transpose (the
    # transposed DMA loads explode into ~1.7M element-granularity descriptors)
    def pe_t(dst_ap, src_ap, k):
        m = src_ap.shape[-1]
        ps = psS.tile([128, N], F32, tag="score")
        nc.tensor.transpose(ps[0:m, 0:k], src_ap, I128[0:k, 0:k])
        nc.scalar.activation(out=dst_ap, in_=ps[0:m, 0:k], func=AF.Copy)

    w5T = [wp.tile([128, 1024], F32, tag=f"w5T{ci}", name=f"w5T{ci}")
           for ci in range(4)]
    for r in range(8):
        st = work.tile([128, 512], F32, tag="wstage")
        nc.sync.dma_start(out=st[:], in_=w_d["w5"][r * 128:(r + 1) * 128, :])
        for c in range(4):
            pe_t(w5T[c][:, r * 128:(r + 1) * 128], st[:, c * 128:(c + 1) * 128], 128)
    l1wT = [wp.tile([128, 512], F32, tag=f"l1wT{ci}", name=f"l1wT{ci}")
            for ci in range(16)]
    for r in range(4):
        for hf in range(4):
            st = work.tile([128, 512], F32, tag="wstage")
            nc.sync.dma_start(out=st[:], in_=w_d["l1w"][r * 128:(r + 1) * 128,
                                                        hf * 512:(hf + 1) * 512])
            for c in range(4):
                pe_t(l1wT[hf * 4 + c][:, r * 128:(r + 1) * 128],
                     st[:, c * 128:(c + 1) * 128], 128)
    l2wT = [wp.tile([128, 256], F32, tag=f"l2wT{ci}", name=f"l2wT{ci}")
            for ci in range(4)]
    for r in range(2):
        st = work.tile([128, 512], F32, tag="wstage")
        nc.sync.dma_start(out=st[:], in_=w_d["l2w"][r * 128:(r + 1) * 128, :])
        for c in range(4):
            pe_t(l2wT[c][:, r * 128:(r + 1) * 128], st[:, c * 128:(c + 1) * 128], 128)
    l3wT = [wp.tile([128, 40], F32, tag=f"l3wT{ci}", name=f"l3wT{ci}")
            for ci in range(2)]
    st = work.tile([128, 512], F32, tag="wstage")
    nc.sync.dma_start(out=st[0:40, 0:256], in_=w_d["l3w"][:])
    for c in range(2):
        pe_t(l3wT[c][:], st[0:40, c * 128:(c + 1) * 128], 40)

    # bn affines
    bnAB = {}
    for i, c in zip(range(1, 5), [64, 64, 128, 256]):
        nch = (c + 127) // 128
        As, Bs = [], []
        for ch in range(nch):
            cc = min(128, c - ch * 128)
            bnT = wp.tile([cc, 4], F32, tag=f"bnT{i}_{ch}")
            tload(bnT[:], w_d["bn%d" % i][:, ch * 128:ch * 128 + cc].rearrange("f c -> c f"))
            A, B = _bn_affine(nc, wp, bnT, f"bn{i}_{ch}")
            As.append(A); Bs.append(B)
        bnAB[i] = (As, Bs)
    A5 = wp.tile([128, 8], F32, tag="A5")
    B5 = wp.tile([128, 8], F32, tag="B5")
    for ch in range(8):
        bnT = wp.tile([128, 4], F32, tag=f"bnT5_{ch}")
        tload(bnT[:], w_d["bn5"][:, ch * 128:(ch + 1) * 128].rearrange("f c -> c f"))
        A, B = _bn_affine(nc, wp, bnT, f"bn5_{ch}")
        nc.vector.tensor_copy(out=A5[:, ch:ch + 1], in_=A[:])
        nc.vector.tensor_copy(out=B5[:, ch:ch + 1], in_=B[:])
    A6 = wp.tile([128, 4], F32, tag="A6")
    B6 = wp.tile([128, 4], F32, tag="B6")
    for ch in range(4):
        bnT = wp.tile([128, 4], F32, tag=f"bnT6_{ch}")
        tload(bnT[:], w_d["bn6"][:, ch * 128:(ch + 1) * 128].rearrange("f c -> c f"))
        A, B = _bn_affine(nc, wp, bnT, f"bn6_{ch}")
        nc.vector.tensor_copy(out=A6[:, ch:ch + 1], in_=A[:])
        nc.vector.tensor_copy(out=B6[:, ch:ch + 1], in_=B[:])
    A7 = wp.tile([128, 2], F32, tag="A7")
    B7 = wp.tile([128, 2], F32, tag="B7")
    for ch in range(2):
        bnT = wp.tile([128, 4], F32, tag=f"bnT7_{ch}")
        tload(bnT[:], w_d["bn7"][:, ch * 128:(ch + 1) * 128].rearrange("f c -> c f"))
        A, B = _bn_affine(nc, wp, bnT, f"bn7_{ch}")
        # fold l2b: B7' = A7*l2b + B7
        l2bT = wp.tile([128, 1], F32, tag=f"l2bT{ch}")
        tload(l2bT[:], w_d["l2b"][ch * 128:(ch + 1) * 128].rearrange("(p o) -> p o", o=1))
        t = wp.tile([128, 1], F32, tag=f"b7f{ch}")
        nc.vector.tensor_tensor(out=t[:], in0=A[:], in1=l2bT[:], op=ALU.mult)
        nc.vector.tensor_tensor(out=t[:], in0=B[:], in1=t[:], op=ALU.add)
        nc.vector.tensor_copy(out=A7[:, ch:ch + 1], in_=A[:])
        nc.vector.tensor_copy(out=B7[:, ch:ch + 1], in_=t[:])
    l3bT = wp.tile([40, 1], F32, tag="l3bT")
    tload(l3bT[:], w_d["l3b"][:].rearrange("(p o) -> p o", o=1))

    # ---------------- per-sample feature tiles ----------------
    # cat layout per sample: catA rows 0:64 = h1, 64:128 = h2; catB = h3;
    # catC/catD = h4 chunks.  All [128, 1024].
    cats = []
    for s in range(2):
        cats.append([cat.tile([128, N], F32, tag=f"cat{t}_{s}", name=f"cat{t}_{s}") for t in "ABCD"])
    xT = []
    for s in range(2):
        t = cat.tile([3, N], F32, tag=f"xT{s}")
        nc.sync.dma_start(out=t[:], in_=x_d[s])
        xT.append(t)

    pooledT = cat.tile([128, 32], F32, tag="pooledT")

    for s in range(2):
        catA, catB, catC, catD = cats[s]
        fT_in = [xT[s][:], catA[0:64, :], catA[64:128, :], catB[:]]
        out_rows = [[catA[0:64, :]], [catA[64:128, :]], [catB[:]], [catC[:], catD[:]]]
        for li, (C, O) in enumerate(LAYERS):
            edge_conv_layer(nc, tc, work, feat, small, psS, psX, psU, dram,
                            s, li, C, O, fT_in[li], gp, w1nP,
                            wn[li], wxm[li], bnAB[li + 1],
                            out_rows[li], iota, ones, cm05)

        # ----- layer 5: 1024-wide conv + pooling -----
        catchunks = [catA, catB, catC, catD]
        for j in range(8):
            h5_ps = psS.tile([128, N], F32, tag="score")
            for ci in range(4):
                for f in range(0, N, 512):
                    nc.tensor.matmul(h5_ps[:, f:f + 512],
                                     w5T[ci][:, j * 128:(j + 1) * 128],
                                     catchunks[ci][:, f:f + 512],
                                     start=(ci == 0), stop=(ci == 3))
            h5_sb = work.tile([128, N], F32, tag="h5")
            sums = small.tile([128, 1], F32, tag="h5sum")
            nc.scalar.activation(out=h5_sb[:], in_=h5_ps[:], func=AF.Prelu,
                                 bias=B5[:, j:j + 1], scale=A5[:, j:j + 1],
                                 alpha=0.2, accum_out=sums[:])
            # mean -> pooled col (8+j)*2+s ; max -> pooled col j*2+s
            nc.scalar.activation(out=pooledT[:, (8 + j) * 2 + s:(8 + j) * 2 + s + 1],
                                 in_=sums[:], func=AF.Copy, scale=1.0 / N)
            nc.vector.tensor_reduce(out=pooledT[:, j * 2 + s:j * 2 + s + 1],
                                    in_=h5_sb[:], axis=AX.X, op=ALU.max)

    # ---------------- MLP head (both samples as free dim) ----------------
    h6T = work.tile([128, 4, 2], F32, tag="h6T")
    for j in range(4):
        h6_ps = psU.tile([128, 2], F32, tag="uv")
        for ci in range(16):
            nc.tensor.matmul(h6_ps[:], l1wT[ci][:, j * 128:(j + 1) * 128],
                             pooledT[:, ci * 2:ci * 2 + 2],
                             start=(ci == 0), stop=(ci == 15))
        nc.scalar.activation(out=h6T[:, j, :], in_=h6_ps[:], func=AF.Prelu,
                             bias=B6[:, j:j + 1], scale=A6[:, j:j + 1], alpha=0.2)
    h7T = work.tile([128, 2, 2], F32, tag="h7T")
    for j in range(2):
        h7_ps = psU.tile([128, 2], F32, tag="uv")
        for ci in range(4):
            nc.tensor.matmul(h7_ps[:], l2wT[ci][:, j * 128:(j + 1) * 128],
                             h6T[:, ci, :], start=(ci == 0), stop=(ci == 3))
        nc.scalar.activation(out=h7T[:, j, :], in_=h7_ps[:], func=AF.Prelu,
                             bias=B7[:, j:j + 1], scale=A7[:, j:j + 1], alpha=0.2)
    out_ps = psU.tile([40, 2], F32, tag="uv")
    for ci in range(2):
        nc.tensor.matmul(out_ps[:], l3wT[ci][:], h7T[:, ci, :],
                         start=(ci == 0), stop=(ci == 1))
    out_sb = small.tile([40, 2], F32, tag="out")
    nc.vector.tensor_scalar(out=out_sb[:], in0=out_ps[:], scalar1=l3bT[:],
                            scalar2=None, op0=ALU.add)
    nc.sync.dma_start(out=out_d[:], in_=out_sb[:])


def edge_conv_layer(nc, tc, work, feat, small, psS, psX, psU, dram,
                    s, li, C, O, fT, gp, w1nP, wnT, wxmT, bnab,
                    out_rows, iota, ones, cm05):
    As, Bs = bnab
    noc = (O + 127) // 128

    # xx = sum_c f^2 (via ones-matmul over partitions)
    sq = work.tile([C, N], F32, tag="sq")
    nc.vector.tensor_tensor(out=sq[:], in0=fT, in1=fT, op=ALU.mult)
    xx_ps = psX.tile([1, N], F32, tag="xx")
    for f in range(0, N, 512):
        nc.tensor.matmul(xx_ps[:, f:f + 512], ones[0:C, :], sq[:, f:f + 512],
                         start=True, stop=True)

    if C < 128:
        AUGP = 32 if C < 32 else C
        rhs_aug = work.tile([AUGP + 1, N], F32, tag="rhsaug")
        lhs_aug = work.tile([AUGP + 1, N], F32, tag="lhsaug")
        if AUGP != C:
            nc.vector.memset(rhs_aug[:], 0.0)
            nc.vector.memset(lhs_aug[:], 0.0)
        nc.scalar.activation(out=rhs_aug[0:C, :], in_=fT, func=AF.Copy)
        nc.scalar.activation(out=rhs_aug[AUGP:AUGP + 1, :], in_=xx_ps[:], func=AF.Copy)
        nc.scalar.activation(out=lhs_aug[0:C, :], in_=fT, func=AF.Copy)
        nc.vector.memset(lhs_aug[AUGP:AUGP + 1, :], -0.5)
        xx_sb = None
    else:
        rhs_aug = lhs_aug = None
        xx_sb = work.tile([1, N], F32, tag="xxsb")
        nc.scalar.activation(out=xx_sb[:], in_=xx_ps[:], func=AF.Copy)

    # u = Wn @ f, v = (Wx-Wn) @ f   (transposed [O, N]); rhs must share the
    # lhsT base partition, so use the base-0 copy in rhs_aug when C < 128.
    fT0 = rhs_aug[0:C, :] if C < 128 else fT
    uT_sb, vT_sb = [], []
    for oc in range(noc):
        ocw = min(128, O - oc * 128)
        if li == 1:
            ups = psU.tile([ocw, N], F32, tag="uv")
            for f in range(0, N, 512):
                nc.tensor.matmul(ups[:, f:f + 512], wnT[:, oc * 128:oc * 128 + ocw],
                                 fT0[:, f:f + 512], start=True, stop=True)
            ut = feat.tile([ocw, N], F32, tag=f"u{oc}")
            nc.scalar.activation(out=ut[:], in_=ups[:], func=AF.Copy)
            uT_sb.append(ut)
        else:
            uT_sb.append(None)
        vps = psU.tile([ocw, N], F32, tag="uv")
        for f in range(0, N, 512):
            nc.tensor.matmul(vps[:, f:f + 512], wxmT[:, oc * 128:oc * 128 + ocw],
                             fT0[:, f:f + 512], start=True, stop=True)
        vt = feat.tile([ocw, N], F32, tag=f"v{oc}")
        nc.scalar.activation(out=vt[:], in_=vps[:], func=AF.Copy)
        vT_sb.append(vt)

    aggT = [feat.tile([min(128, O - oc * 128), N], F32, tag=f"agg{oc}", name=f"agg{oc}")
            for oc in range(noc)]

    for b in range(8):
        # ---- score tile [128, N] ----
        sc_ps = psS.tile([128, N], F32, tag="score")
        for f in range(0, N, 512):
            if C < 128:
                nc.tensor.matmul(sc_ps[:, f:f + 512],
                                 lhs_aug[:, b * 128:(b + 1) * 128],
                                 rhs_aug[:, f:f + 512], start=True, stop=True)
            else:
                nc.tensor.matmul(sc_ps[:, f:f + 512], fT[:, b * 128:(b + 1) * 128],
                                 fT[:, f:f + 512], start=True, stop=False)
                nc.tensor.matmul(sc_ps[:, f:f + 512], cm05[:],
                                 xx_sb[:, f:f + 512], start=False, stop=True)
        # ---- pack + top-20 cascade ----
        packed = work.tile([128, N], U32, tag="packed")
        _stt_u32(nc, packed[:], sc_ps[:].bitcast(U32), 0xFFFFFC00, iota[:],
                 ALU.bitwise_and, ALU.bitwise_or)
        packf = packed[:].bitcast(F32)
        top24 = small.tile([128, 24], F32, tag="top24")
        nc.vector.max(top24[:, 0:8], packf)
        nc.vector.match_replace(packf, top24[:, 0:8], packf, imm_value=NEG)
        nc.vector.max(top24[:, 8:16], packf)
        nc.vector.match_replace(packf, top24[:, 8:16], packf, imm_value=NEG)
        nc.vector.max(top24[:, 16:24], packf)
        idx32 = small.tile([128, 32], U32, tag="idx32")
        _ts_u32(nc, idx32[:, 0:20], top24[:, 0:20].bitcast(U32), 0x3FF,
                ALU.bitwise_and)
        nc.vector.tensor_copy(out=idx32[:, 20:32], in_=idx32[:, 0:12])
        idxP = small.tile([128, 32], U16, tag="idxP")
        nc.vector.tensor_copy(out=idxP[:], in_=idx32[:])
        # ---- wrap roundtrip (3 DMAs) ----
        scrW = dram.tile([4096], U16, tag="scrW")
        nc.sync.dma_start(
            out=scrW[:].rearrange("(j1 n j0) -> n j1 j0", j1=16, j0=2),
            in_=idxP[:])
        scr2 = dram.tile([8, 4096], U16, tag="scr2")
        nc.scalar.dma_start(out=scr2[:].rearrange("a q -> (a q)"),
                            in_=scrW[:].unsqueeze(0).broadcast_to([8, 4096]))
        idxw = small.tile([128, 256], I16, tag="idxw")
        nc.sync.dma_start(
            out=idxw[:],
            in_=scr2[:].bitcast(I16).rearrange("a q -> (a q)").rearrange("(p q) -> p q", p=128))
        # ---- gather + reduce (full 128-point block per call) ----
        if li != 1:
            # gather the (narrower) input f once, apply Wn per edge on the PE
            if li == 0:
                Cg, gsrc, wlhs = 16, rhs_aug[0:16, :], w1nP[:]
            elif li == 2:
                Cg, gsrc, wlhs = 64, fT0, wnT
            else:
                Cg, gsrc, wlhs = 128, fT, wnT
            g = gp.tile([Cg, 4096], F32, tag="g")
            nc.gpsimd.ap_gather(g[:], gsrc, idxw[0:Cg, 0:256],
                                channels=Cg, num_elems=N, d=1, num_idxs=4096)
            for oc in range(noc):
                ocw = min(128, O - oc * 128)
                for c in range(8):
                    ue = psS.tile([128, N], F32, tag="score")
                    nc.tensor.matmul(ue[0:ocw, 0:512],
                                     wlhs[:, oc * 128:oc * 128 + ocw],
                                     g[:, c * 512:(c + 1) * 512],
                                     start=True, stop=True)
                    uev = ue[0:ocw, 0:512].rearrange(
                        "o (n run pos) -> o n run pos", run=2, pos=16)[:, :, :, 0:10]
                    nc.vector.tensor_reduce(
                        out=aggT[oc][:, b * 128 + c * 16:b * 128 + (c + 1) * 16],
                        in_=uev, axis=AX.XY, op=ALU.max)
        else:
            for oc in range(noc):
                ocw = min(128, O - oc * 128)
                g = gp.tile([ocw, 4096], F32, tag="g")
                nc.gpsimd.ap_gather(g[:], uT_sb[oc][:], idxw[0:ocw, 0:256],
                                    channels=ocw, num_elems=N, d=1, num_idxs=4096)
                gv = g[:].rearrange("o (n run pos) -> o n run pos", run=2, pos=16)[:, :, :, 0:10]
                nc.vector.tensor_reduce(
                    out=aggT[oc][:, b * 128:(b + 1) * 128],
                    in_=gv, axis=AX.XY, op=ALU.max)

    # ---- h = lrelu(A*(agg + v) + B) -> cat rows ----
    for oc in range(noc):
        nc.vector.tensor_tensor(out=aggT[oc][:], in0=aggT[oc][:], in1=vT_sb[oc][:],
                                op=ALU.add)
        nc.scalar.activation(out=out_rows[oc], in_=aggT[oc][:], func=AF.Prelu,
                             bias=Bs[oc][:], scale=As[oc][:], alpha=0.2)


_NC_CACHE = []


def kernel(**inputs):
    """Full-batch entry: shard 16 samples over 8 cores (2 each), run SPMD."""
    from concourse.bass_utils import run_bass_kernel_spmd

    if not _NC_CACHE:
        _NC_CACHE.append(build_nc())
    nc = _NC_CACHE[0]

    x = np.ascontiguousarray(inputs["x"], dtype=np.float32)
    base = {k: np.ascontiguousarray(v, dtype=np.float32)
            for k, v in inputs.items() if k != "x"}
    cores = list(range(8))
    in_maps = [dict(base, x=np.ascontiguousarray(x[2 * c:2 * c + 2])) for c in cores]
    res = run_bass_kernel_spmd(nc, in_maps, cores).results
    out = np.concatenate([np.ascontiguousarray(r["outT"]).T for r in res], axis=0)
    return out.astype(np.float32)
